# revision 6
# baseline (speedup 1.0000x reference)
"""Multi-head attention + out-proj + residual + LayerNorm on 8 trn2 cores.

Sharding: (batch, seq-half) -> 8 shards, collective-free. Each core gets
transposed activations (host-prepped) plus shared (transposed) weights and
computes its full [1024, 1024] output block:

  phase V: V_all[Sk, H, dv]   = vT.T per-head proj  (kept in SBUF, +ones col)
  phase K: KT_all[H*dk, Sk]   -> staged to DRAM
  phase Q: QT_all[H*dk, Sq]   (kept in SBUF)
  attn  : per head: scoresT[Sk,Sq] = KT_h.T@QT_h -> exp(x/sqrt(D)) (ACT)
          OT[dv+1, Sq] += [V_h|1].T @ expT   (row 64 = softmax denom)
          OT[0:64] *= bcast(1/denom)  -> staged to DRAM (concat.T layout)
  final : out = LN(concatT.T @ WpT + q_res) * scale + offset
"""

import os
from contextlib import ExitStack

import numpy as np

import concourse.bass as bass
import concourse.tile as tile
from concourse import bacc, mybir
from concourse._compat import with_exitstack
from concourse.bass_utils import run_bass_kernel_spmd

B, S, D = 4, 2048, 1024
H, DK, DV = 16, 64, 64
F = H * DV            # 1024 flattened head dim (== H*DK)
N_CORES = 8
SQ = S // 2           # 1024 queries per core
SK = S                # 2048 keys per core
P = 128
KD = D // P           # 8 contraction chunks over d_model
NF = F // P           # 8 head-pair chunks
NSK = SK // P         # 16 key chunks
TEMP = float(np.sqrt(D))
EPS = 1e-9

F32 = mybir.dt.float32

LAST_RESULT = None    # BassKernelResults of the most recent kernel() call


@with_exitstack
def _mha_kernel(ctx: ExitStack, tc: tile.TileContext, out_ap, ins):
    nc = tc.nc
    AF = mybir.ActivationFunctionType
    ALU = mybir.AluOpType

    g_const = ctx.enter_context(tc.tile_pool(name="gconst", bufs=1))
    dram = ctx.enter_context(tc.tile_pool(name="dramstage", bufs=1, space="DRAM"))

    kt_stage = dram.tile([NF, P, SK], F32)   # KT_all, head-pair-chunk major
    ot_stage = dram.tile([NF, P, SQ], F32)   # concat.T, head-pair-chunk major

    ones_sb = g_const.tile([P, 64], F32)
    nc.vector.memset(ones_sb, 1.0)

    xq_r = ins["qT"].rearrange("(c p) s -> p c s", p=P)
    xk_r = ins["kT"].rearrange("(c p) s -> p c s", p=P)
    xv_r = ins["vT"].rearrange("(c p) s -> p c s", p=P)

    with (
        tc.tile_pool(name="resident", bufs=1) as resident,
        tc.tile_pool(name="wts", bufs=2) as wpool,
    ):
        # V_all with a ones column appended per head: [sk_part, sk, head, 65]
        v_sb = resident.tile([P, NSK, H, 65], F32)
        qt_sb = resident.tile([P, NF, SQ], F32)
        nc.vector.memset(v_sb[:, :, :, 64:65], 1.0)

        # ---------------- V projection: V_all[Sk, F] (natural layout) ------
        wv = wpool.tile([P, KD, F], F32, tag="w")
        nc.sync.dma_start(wv, ins["wvT"].rearrange("(c p) f -> p c f", p=P))
        with (
            tc.tile_pool(name="xv", bufs=12) as xpool,
            tc.tile_pool(name="vps", bufs=4, space="PSUM") as vps,
        ):
            for sk in range(NSK):
                xts = []
                for kd in range(KD):
                    t = xpool.tile([P, P], F32, tag="x", name="xt")
                    nc.sync.dma_start(t, xv_r[:, kd, sk * P:(sk + 1) * P])
                    xts.append(t)
                pss = [vps.tile([P, 512], F32, tag="ps", name="vp") for _ in range(2)]
                for kd in range(KD):
                    for n in range(2):
                        nc.tensor.matmul(
                            pss[n],
                            lhsT=xts[kd],
                            rhs=wv[:, kd, n * 512:(n + 1) * 512],
                            start=(kd == 0),
                            stop=(kd == KD - 1),
                        )
                for n in range(2):
                    nc.scalar.copy(
                        v_sb[:, sk, n * 8:(n + 1) * 8, 0:64],
                        pss[n].rearrange("p (h e) -> p h e", h=8),
                    )

        # ---------------- K projection: KT_all[F, Sk] -> DRAM --------------
        wk = wpool.tile([P, KD, F], F32, tag="w")
        nc.sync.dma_start(wk, ins["wkT"].rearrange("(c p) f -> p c f", p=P))
        with (
            tc.tile_pool(name="xk", bufs=8) as xpool,
            tc.tile_pool(name="kout", bufs=4) as kout,
            tc.tile_pool(name="kps", bufs=4, space="PSUM") as kps,
        ):
            for n in range(SK // 512):  # 4 key blocks
                xts = []
                for kd in range(KD):
                    t = xpool.tile([P, 512], F32, tag="x", name="xt")
                    nc.sync.dma_start(t, xk_r[:, kd, n * 512:(n + 1) * 512])
                    xts.append(t)
                for f in range(NF):
                    ps = kps.tile([P, 512], F32, tag="ps")
                    for kd in range(KD):
                        nc.tensor.matmul(
                            ps,
                            lhsT=wk[:, kd, f * P:(f + 1) * P],
                            rhs=xts[kd],
                            start=(kd == 0),
                            stop=(kd == KD - 1),
                        )
                    ko = kout.tile([P, 512], F32, tag="ko")
                    nc.scalar.copy(ko, ps)
                    nc.sync.dma_start(kt_stage[f, :, n * 512:(n + 1) * 512], ko)

        # ---------------- Q projection: QT_all[F, Sq] -> SBUF --------------
        wq = wpool.tile([P, KD, F], F32, tag="w")
        nc.sync.dma_start(wq, ins["wqT"].rearrange("(c p) f -> p c f", p=P))
        with (
            tc.tile_pool(name="xq", bufs=8) as xpool,
            tc.tile_pool(name="qps", bufs=4, space="PSUM") as qps,
        ):
            for n in range(SQ // 512):  # 2 query blocks
                xts = []
                for kd in range(KD):
                    t = xpool.tile([P, 512], F32, tag="x", name="xt")
                    nc.sync.dma_start(t, xq_r[:, kd, n * 512:(n + 1) * 512])
                    xts.append(t)
                for f in range(NF):
                    ps = qps.tile([P, 512], F32, tag="ps")
                    for kd in range(KD):
                        nc.tensor.matmul(
                            ps,
                            lhsT=wq[:, kd, f * P:(f + 1) * P],
                            rhs=xts[kd],
                            start=(kd == 0),
                            stop=(kd == KD - 1),
                        )
                    nc.scalar.copy(qt_sb[:, f, n * 512:(n + 1) * 512], ps)

        # ---------------- attention, head by head --------------------------
        with (
            tc.tile_pool(name="ktc", bufs=2) as ktp,
            tc.tile_pool(name="expp", bufs=2) as expp,
            tc.tile_pool(name="rcp", bufs=2) as rcp,
            tc.tile_pool(name="bcs", bufs=2) as bcs,
            tc.tile_pool(name="oto", bufs=2) as oto,
            tc.tile_pool(name="scps", bufs=1, space="PSUM") as scps,
            tc.tile_pool(name="smps", bufs=4, space="PSUM") as smps,
        ):
            for c in range(NF):  # head-pair chunks
                ktc = ktp.tile([P, SK], F32, tag="kt")
                nc.sync.dma_start(ktc, kt_stage[c])
                for hh in range(2):
                    h = 2 * c + hh
                    base = hh * 64
                    ot_ps = [
                        smps.tile([65, 512], F32, tag="sm", name="otp")
                        for _ in range(2)
                    ]
                    for g in range(NSK // 2):  # groups of 2 key chunks
                        sc = scps.tile([P, 2, 2, 512], F32, tag="sc")
                        for skj in range(2):
                            sk = 2 * g + skj
                            for sq in range(2):
                                nc.tensor.matmul(
                                    sc[:, skj, sq, :],
                                    lhsT=ktc[base:base + 64, sk * P:(sk + 1) * P],
                                    rhs=qt_sb[base:base + 64, c,
                                              sq * 512:(sq + 1) * 512],
                                    start=True,
                                    stop=True,
                                )
                        ex = expp.tile([P, 2, 2, 512], F32, tag="ex")
                        nc.scalar.activation(ex, sc, AF.Exp, scale=1.0 / TEMP)
                        for skj in range(2):
                            sk = 2 * g + skj
                            for sq in range(2):
                                nc.tensor.matmul(
                                    ot_ps[sq],
                                    lhsT=v_sb[:, sk, h, :],
                                    rhs=ex[:, skj, sq, :],
                                    start=(sk == 0),
                                    stop=(sk == NSK - 1),
                                )
                    for sq in range(2):
                        # row 64 of ot_ps = softmax denominator
                        rc = rcp.tile([65, 512], F32, tag="rc")
                        nc.vector.reciprocal(rc[64:65, :], ot_ps[sq][64:65, :])
                        bc_ps = smps.tile([64, 512], F32, tag="sm")
                        nc.tensor.matmul(
                            bc_ps,
                            lhsT=ones_sb[64:65, 0:64],
                            rhs=rc[64:65, :],
                            start=True,
                            stop=True,
                        )
                        bc = bcs.tile([64, 512], F32, tag="bc")
                        nc.vector.tensor_copy(bc, bc_ps)
                        oo = oto.tile([64, 512], F32, tag="oo")
                        nc.vector.tensor_mul(oo, ot_ps[sq][0:64, :], bc)
                        nc.sync.dma_start(
                            ot_stage[c, base:base + 64, sq * 512:(sq + 1) * 512],
                            oo,
                        )

    # ---------------- output projection + residual + layernorm -------------
    with (
        tc.tile_pool(name="wp", bufs=1) as wpp,
        tc.tile_pool(name="lnc", bufs=1) as lnc,
        tc.tile_pool(name="otf", bufs=12) as otf,
        tc.tile_pool(name="qres", bufs=2) as qrp,
        tc.tile_pool(name="lnw", bufs=2) as lnw,
        tc.tile_pool(name="stat", bufs=4) as stp,
        tc.tile_pool(name="fps", bufs=2, space="PSUM") as fps,
    ):
        wp = wpp.tile([P, NF, D], F32)
        nc.sync.dma_start(wp, ins["wpT"].rearrange("(c p) f -> p c f", p=P))
        scale_sb = lnc.tile([P, 2, 512], F32)
        nc.sync.dma_start(scale_sb, ins["scale_b"].rearrange("p (a b) -> p a b", a=2))
        offset_sb = lnc.tile([P, 2, 512], F32)
        nc.sync.dma_start(offset_sb, ins["offset_b"].rearrange("p (a b) -> p a b", a=2))

        for sq in range(SQ // P):  # 8 query chunks of 128
            ots = []
            for f in range(NF):
                t = otf.tile([P, P], F32, tag="ot", name="ott")
                nc.sync.dma_start(t, ot_stage[f, :, sq * P:(sq + 1) * P])
                ots.append(t)
            qr = qrp.tile([P, 2, 512], F32, tag="qr")
            nc.sync.dma_start(
                qr,
                ins["qres"][sq * P:(sq + 1) * P, :].rearrange(
                    "p (a b) -> p a b", a=2),
            )
            fp = fps.tile([P, 2, 512], F32, tag="fp")
            for d in range(2):
                for f in range(NF):
                    nc.tensor.matmul(
                        fp[:, d, :],
                        lhsT=ots[f],
                        rhs=wp[:, f, d * 512:(d + 1) * 512],
                        start=(f == 0),
                        stop=(f == NF - 1),
                    )
            x = lnw.tile([P, 2, 512], F32, tag="x")
            nc.vector.tensor_add(x, fp, qr)
            stats = stp.tile([P, 2, 6], F32, tag="st")
            for gsub in range(2):
                nc.vector.bn_stats(stats[:, gsub, :], x[:, gsub, :])
            mv = stp.tile([P, 2], F32, tag="mv")
            nc.vector.bn_aggr(mv, stats)
            # unbiased std + eps, then reciprocal
            stdt = stp.tile([P, 1], F32, tag="sd")
            nc.scalar.activation(stdt, mv[:, 1:2], AF.Sqrt,
                                 scale=float(D) / float(D - 1))
            nc.vector.tensor_scalar_add(stdt, stdt, EPS)
            rstd = stp.tile([P, 1], F32, tag="rs")
            nc.vector.reciprocal(rstd, stdt)
            xn = lnw.tile([P, 2, 512], F32, tag="xn")
            nc.vector.tensor_scalar(xn, x, mv[:, 0:1], rstd,
                                    ALU.subtract, ALU.mult)
            nc.vector.tensor_mul(xn, xn, scale_sb)
            nc.vector.tensor_add(xn, xn, offset_sb)
            nc.sync.dma_start(
                out_ap[sq * P:(sq + 1) * P, :],
                xn.rearrange("p a b -> p (a b)"),
            )


def build_program():
    nc = bacc.Bacc("TRN2", debug=False, target_bir_lowering=False)
    shapes = {
        "qT": [D, SQ], "kT": [D, SK], "vT": [D, SK], "qres": [SQ, D],
        "wqT": [D, F], "wkT": [D, F], "wvT": [D, F], "wpT": [F, D],
        "scale_b": [P, D], "offset_b": [P, D],
    }
    ins = {k: nc.dram_tensor(k, v, F32, kind="ExternalInput").ap()
           for k, v in shapes.items()}
    out = nc.dram_tensor("out", [SQ, D], F32, kind="ExternalOutput").ap()
    with tile.TileContext(nc) as tc:
        _mha_kernel(tc, out, ins)
    nc.compile()
    return nc


_PROGRAM = None


def _get_program():
    global _PROGRAM
    if _PROGRAM is None:
        _PROGRAM = build_program()
    return _PROGRAM


def make_in_maps(q, k, v, Wq, Wk, Wv, Wp, scale, offset):
    f = np.float32
    q = np.asarray(q, f)
    k = np.asarray(k, f)
    v = np.asarray(v, f)
    wqT = np.ascontiguousarray(np.asarray(Wq, f).transpose(2, 0, 1).reshape(D, F))
    wkT = np.ascontiguousarray(np.asarray(Wk, f).transpose(2, 0, 1).reshape(D, F))
    wvT = np.ascontiguousarray(np.asarray(Wv, f).transpose(2, 0, 1).reshape(D, F))
    wpT = np.ascontiguousarray(np.asarray(Wp, f).T)
    scale_b = np.ascontiguousarray(
        np.broadcast_to(np.asarray(scale, f), (P, D)))
    offset_b = np.ascontiguousarray(
        np.broadcast_to(np.asarray(offset, f), (P, D)))
    in_maps = []
    for c in range(N_CORES):
        b, half = divmod(c, 2)
        sl = slice(half * SQ, (half + 1) * SQ)
        in_maps.append({
            "qT": np.ascontiguousarray(q[b, sl].T),
            "qres": np.ascontiguousarray(q[b, sl]),
            "kT": np.ascontiguousarray(k[b].T),
            "vT": np.ascontiguousarray(v[b].T),
            "wqT": wqT, "wkT": wkT, "wvT": wvT, "wpT": wpT,
            "scale_b": scale_b, "offset_b": offset_b,
        })
    return in_maps


def kernel(q, k, v, Wq, Wk, Wv, Wp, scale, offset):
    global LAST_RESULT
    in_maps = make_in_maps(q, k, v, Wq, Wk, Wv, Wp, scale, offset)
    nc = _get_program()
    res = run_bass_kernel_spmd(nc, in_maps, list(range(N_CORES)))
    LAST_RESULT = res
    out = np.empty((B, S, D), np.float32)
    for c in range(N_CORES):
        b, half = divmod(c, 2)
        out[b, half * SQ:(half + 1) * SQ] = res.results[c]["out"]
    return out


# revision 8
# speedup vs baseline: 2.1136x; 2.1136x over previous
"""Multi-head attention + out-proj + residual + LayerNorm on 8 trn2 cores.

Sharding: (batch, seq-half) -> 8 shards, collective-free. Each core gets
transposed activations (host-prepped) plus shared (transposed) weights and
computes its full [1024, 1024] output block:

  phase V: V_all[Sk, H, dv]   = vT.T per-head proj  (kept in SBUF, +ones col)
  phase K: KT_all[H*dk, Sk]   -> staged to DRAM
  phase Q: QT_all[H*dk, Sq]   (kept in SBUF)
  attn  : per head: scoresT[Sk,Sq] = KT_h.T@QT_h -> exp(x/sqrt(D)) (ACT)
          OT[dv+1, Sq] += [V_h|1].T @ expT   (row 64 = softmax denom)
          OT[0:64] *= bcast(1/denom)  -> staged to DRAM (concat.T layout)
  final : out = LN(concatT.T @ WpT + q_res) * scale + offset
"""

import os
from contextlib import ExitStack

import numpy as np

import concourse.bass as bass
import concourse.tile as tile
from concourse import bacc, mybir
from concourse._compat import with_exitstack
from concourse.bass_utils import run_bass_kernel_spmd

B, S, D = 4, 2048, 1024
H, DK, DV = 16, 64, 64
F = H * DV            # 1024 flattened head dim (== H*DK)
N_CORES = 8
SQ = S // 2           # 1024 queries per core
SK = S                # 2048 keys per core
P = 128
KD = D // P           # 8 contraction chunks over d_model
NF = F // P           # 8 head-pair chunks
NSK = SK // P         # 16 key chunks
TEMP = float(np.sqrt(D))
EPS = 1e-9

F32 = mybir.dt.float32
BF16 = mybir.dt.bfloat16
F32R = mybir.dt.float32r

LAST_RESULT = None    # BassKernelResults of the most recent kernel() call


@with_exitstack
def _mha_kernel(ctx: ExitStack, tc: tile.TileContext, out_ap, ins):
    nc = tc.nc
    AF = mybir.ActivationFunctionType
    ALU = mybir.AluOpType

    g_const = ctx.enter_context(tc.tile_pool(name="gconst", bufs=1))
    dram = ctx.enter_context(tc.tile_pool(name="dramstage", bufs=1, space="DRAM"))

    kt_stage = dram.tile([NF, P, SK], BF16)   # KT_all, head-pair-chunk major
    ot_stage = dram.tile([NF, P, SQ], BF16)   # concat.T, head-pair-chunk major

    ones_sb = g_const.tile([P, 64], BF16)
    nc.vector.memset(ones_sb, 1.0)

    xq_r = ins["qT"].rearrange("(c p) s -> p c s", p=P)
    xk_r = ins["kT"].rearrange("(c p) s -> p c s", p=P)
    xv_r = ins["vT"].rearrange("(c p) s -> p c s", p=P)

    with (
        tc.tile_pool(name="resident", bufs=1) as resident,
        tc.tile_pool(name="wts", bufs=2) as wpool,
    ):
        # V_all with a ones column appended per head: [sk_part, sk, head, 65]
        v_sb = resident.tile([P, NSK, H, 65], BF16)
        qt_sb = resident.tile([P, NF, SQ], BF16)
        nc.vector.memset(v_sb[:, :, :, 64:65], 1.0)

        # ---------------- V projection: V_all[Sk, F] (natural layout) ------
        wv = wpool.tile([P, KD, F], BF16, tag="w")
        nc.sync.dma_start(wv, ins["wvT"].rearrange("(c p) f -> p c f", p=P))
        with (
            tc.tile_pool(name="xv", bufs=12) as xpool,
            tc.tile_pool(name="vps", bufs=4, space="PSUM") as vps,
        ):
            for sk in range(NSK):
                xts = []
                for kd in range(KD):
                    t = xpool.tile([P, P], BF16, tag="x", name="xt")
                    nc.sync.dma_start(t, xv_r[:, kd, sk * P:(sk + 1) * P])
                    xts.append(t)
                pss = [vps.tile([P, 512], F32, tag="ps", name="vp") for _ in range(2)]
                for kd in range(KD):
                    for n in range(2):
                        nc.tensor.matmul(
                            pss[n],
                            lhsT=xts[kd],
                            rhs=wv[:, kd, n * 512:(n + 1) * 512],
                            start=(kd == 0),
                            stop=(kd == KD - 1),
                        )
                for n in range(2):
                    nc.scalar.copy(
                        v_sb[:, sk, n * 8:(n + 1) * 8, 0:64],
                        pss[n].rearrange("p (h e) -> p h e", h=8),
                    )

        # ---------------- K projection: KT_all[F, Sk] -> DRAM --------------
        wk = wpool.tile([P, KD, F], BF16, tag="w")
        nc.sync.dma_start(wk, ins["wkT"].rearrange("(c p) f -> p c f", p=P))
        with (
            tc.tile_pool(name="xk", bufs=8) as xpool,
            tc.tile_pool(name="kout", bufs=4) as kout,
            tc.tile_pool(name="kps", bufs=4, space="PSUM") as kps,
        ):
            for n in range(SK // 512):  # 4 key blocks
                xts = []
                for kd in range(KD):
                    t = xpool.tile([P, 512], BF16, tag="x", name="xt")
                    nc.sync.dma_start(t, xk_r[:, kd, n * 512:(n + 1) * 512])
                    xts.append(t)
                for f in range(NF):
                    ps = kps.tile([P, 512], F32, tag="ps")
                    for kd in range(KD):
                        nc.tensor.matmul(
                            ps,
                            lhsT=wk[:, kd, f * P:(f + 1) * P],
                            rhs=xts[kd],
                            start=(kd == 0),
                            stop=(kd == KD - 1),
                        )
                    ko = kout.tile([P, 512], BF16, tag="ko")
                    nc.scalar.copy(ko, ps)
                    nc.sync.dma_start(kt_stage[f, :, n * 512:(n + 1) * 512], ko)

        # ---------------- Q projection: QT_all[F, Sq] -> SBUF --------------
        wq = wpool.tile([P, KD, F], BF16, tag="w")
        nc.sync.dma_start(wq, ins["wqT"].rearrange("(c p) f -> p c f", p=P))
        with (
            tc.tile_pool(name="xq", bufs=8) as xpool,
            tc.tile_pool(name="qps", bufs=4, space="PSUM") as qps,
        ):
            for n in range(SQ // 512):  # 2 query blocks
                xts = []
                for kd in range(KD):
                    t = xpool.tile([P, 512], BF16, tag="x", name="xt")
                    nc.sync.dma_start(t, xq_r[:, kd, n * 512:(n + 1) * 512])
                    xts.append(t)
                for f in range(NF):
                    ps = qps.tile([P, 512], F32, tag="ps")
                    for kd in range(KD):
                        nc.tensor.matmul(
                            ps,
                            lhsT=wq[:, kd, f * P:(f + 1) * P],
                            rhs=xts[kd],
                            start=(kd == 0),
                            stop=(kd == KD - 1),
                        )
                    nc.scalar.copy(qt_sb[:, f, n * 512:(n + 1) * 512], ps)

        # ---------------- attention, head by head --------------------------
        with (
            tc.tile_pool(name="ktc", bufs=2) as ktp,
            tc.tile_pool(name="expp", bufs=2) as expp,
            tc.tile_pool(name="rcp", bufs=2) as rcp,
            tc.tile_pool(name="bcs", bufs=2) as bcs,
            tc.tile_pool(name="oto", bufs=2) as oto,
            tc.tile_pool(name="scps", bufs=1, space="PSUM") as scps,
            tc.tile_pool(name="smps", bufs=4, space="PSUM") as smps,
        ):
            for c in range(NF):  # head-pair chunks
                ktc = ktp.tile([P, SK], BF16, tag="kt")
                nc.sync.dma_start(ktc, kt_stage[c])
                for hh in range(2):
                    h = 2 * c + hh
                    base = hh * 64
                    ot_ps = [
                        smps.tile([65, 512], F32, tag="sm", name="otp")
                        for _ in range(2)
                    ]
                    for g in range(NSK // 2):  # groups of 2 key chunks
                        sc = scps.tile([P, 2, 2, 512], F32, tag="sc")
                        for skj in range(2):
                            sk = 2 * g + skj
                            for sq in range(2):
                                nc.tensor.matmul(
                                    sc[:, skj, sq, :],
                                    lhsT=ktc[base:base + 64, sk * P:(sk + 1) * P],
                                    rhs=qt_sb[base:base + 64, c,
                                              sq * 512:(sq + 1) * 512],
                                    start=True,
                                    stop=True,
                                )
                        ex = expp.tile([P, 2, 2, 512], BF16, tag="ex")
                        nc.scalar.activation(ex, sc, AF.Exp, scale=1.0 / TEMP)
                        for skj in range(2):
                            sk = 2 * g + skj
                            for sq in range(2):
                                nc.tensor.matmul(
                                    ot_ps[sq],
                                    lhsT=v_sb[:, sk, h, :],
                                    rhs=ex[:, skj, sq, :],
                                    start=(sk == 0),
                                    stop=(sk == NSK - 1),
                                )
                    for sq in range(2):
                        # row 64 of ot_ps = softmax denominator
                        rc = rcp.tile([65, 512], F32, tag="rc")
                        nc.vector.reciprocal(rc[64:65, :], ot_ps[sq][64:65, :])
                        rcb = rcp.tile([65, 512], BF16, tag="rcb")
                        nc.vector.tensor_copy(rcb[64:65, :], rc[64:65, :])
                        bc_ps = smps.tile([64, 512], F32, tag="sm")
                        nc.tensor.matmul(
                            bc_ps,
                            lhsT=ones_sb[64:65, 0:64],
                            rhs=rcb[64:65, :],
                            start=True,
                            stop=True,
                        )
                        bc = bcs.tile([64, 512], F32, tag="bc")
                        nc.vector.tensor_copy(bc, bc_ps)
                        oo = oto.tile([64, 512], BF16, tag="oo")
                        nc.vector.tensor_mul(oo, ot_ps[sq][0:64, :], bc)
                        nc.sync.dma_start(
                            ot_stage[c, base:base + 64, sq * 512:(sq + 1) * 512],
                            oo,
                        )

    # ---------------- output projection + residual + layernorm -------------
    with (
        tc.tile_pool(name="wp", bufs=1) as wpp,
        tc.tile_pool(name="lnc", bufs=1) as lnc,
        tc.tile_pool(name="otf", bufs=12) as otf,
        tc.tile_pool(name="qres", bufs=2) as qrp,
        tc.tile_pool(name="lnw", bufs=2) as lnw,
        tc.tile_pool(name="stat", bufs=4) as stp,
        tc.tile_pool(name="fps", bufs=2, space="PSUM") as fps,
    ):
        wp = wpp.tile([P, NF, D], BF16)
        nc.sync.dma_start(wp, ins["wpT"].rearrange("(c p) f -> p c f", p=P))
        scale_sb = lnc.tile([P, 2, 512], F32)
        nc.sync.dma_start(scale_sb, ins["scale_b"].rearrange("p (a b) -> p a b", a=2))
        offset_sb = lnc.tile([P, 2, 512], F32)
        nc.sync.dma_start(offset_sb, ins["offset_b"].rearrange("p (a b) -> p a b", a=2))

        for sq in range(SQ // P):  # 8 query chunks of 128
            ots = []
            for f in range(NF):
                t = otf.tile([P, P], BF16, tag="ot", name="ott")
                nc.sync.dma_start(t, ot_stage[f, :, sq * P:(sq + 1) * P])
                ots.append(t)
            qr = qrp.tile([P, 2, 512], F32, tag="qr")
            nc.sync.dma_start(
                qr,
                ins["qres"][sq * P:(sq + 1) * P, :].rearrange(
                    "p (a b) -> p a b", a=2),
            )
            fp = fps.tile([P, 2, 512], F32, tag="fp")
            for d in range(2):
                for f in range(NF):
                    nc.tensor.matmul(
                        fp[:, d, :],
                        lhsT=ots[f],
                        rhs=wp[:, f, d * 512:(d + 1) * 512],
                        start=(f == 0),
                        stop=(f == NF - 1),
                    )
            x = lnw.tile([P, 2, 512], F32, tag="x")
            nc.vector.tensor_add(x, fp, qr)
            stats = stp.tile([P, 2, 6], F32, tag="st")
            for gsub in range(2):
                nc.vector.bn_stats(stats[:, gsub, :], x[:, gsub, :])
            mv = stp.tile([P, 2], F32, tag="mv")
            nc.vector.bn_aggr(mv, stats)
            # unbiased std + eps, then reciprocal
            stdt = stp.tile([P, 1], F32, tag="sd")
            nc.scalar.activation(stdt, mv[:, 1:2], AF.Sqrt,
                                 scale=float(D) / float(D - 1))
            nc.vector.tensor_scalar_add(stdt, stdt, EPS)
            rstd = stp.tile([P, 1], F32, tag="rs")
            nc.vector.reciprocal(rstd, stdt)
            xn = lnw.tile([P, 2, 512], F32, tag="xn")
            nc.vector.tensor_scalar(xn, x, mv[:, 0:1], rstd,
                                    ALU.subtract, ALU.mult)
            nc.vector.tensor_mul(xn, xn, scale_sb)
            nc.vector.tensor_add(xn, xn, offset_sb)
            nc.sync.dma_start(
                out_ap[sq * P:(sq + 1) * P, :],
                xn.rearrange("p a b -> p (a b)"),
            )


def build_program():
    nc = bacc.Bacc("TRN2", debug=False, target_bir_lowering=False)
    shapes = {
        "qT": ([D, SQ], BF16), "kT": ([D, SK], BF16), "vT": ([D, SK], BF16),
        "qres": ([SQ, D], F32),
        "wqT": ([D, F], BF16), "wkT": ([D, F], BF16), "wvT": ([D, F], BF16),
        "wpT": ([F, D], BF16),
        "scale_b": ([P, D], F32), "offset_b": ([P, D], F32),
    }
    ins = {k: nc.dram_tensor(k, shp, dt, kind="ExternalInput").ap()
           for k, (shp, dt) in shapes.items()}
    out = nc.dram_tensor("out", [SQ, D], F32, kind="ExternalOutput").ap()
    with tile.TileContext(nc) as tc:
        _mha_kernel(tc, out, ins)
    nc.compile()
    return nc


_PROGRAM = None


def _get_program():
    global _PROGRAM
    if _PROGRAM is None:
        _PROGRAM = build_program()
    return _PROGRAM


def make_in_maps(q, k, v, Wq, Wk, Wv, Wp, scale, offset):
    import ml_dtypes
    f = np.float32
    bf = ml_dtypes.bfloat16
    q = np.asarray(q, f)
    k16 = np.asarray(k, f).astype(bf)
    v16 = np.asarray(v, f).astype(bf)
    q16 = q.astype(bf)
    wqT = np.ascontiguousarray(
        np.asarray(Wq, f).transpose(2, 0, 1).reshape(D, F).astype(bf))
    wkT = np.ascontiguousarray(
        np.asarray(Wk, f).transpose(2, 0, 1).reshape(D, F).astype(bf))
    wvT = np.ascontiguousarray(
        np.asarray(Wv, f).transpose(2, 0, 1).reshape(D, F).astype(bf))
    wpT = np.ascontiguousarray(np.asarray(Wp, f).T.astype(bf))
    scale_b = np.ascontiguousarray(
        np.broadcast_to(np.asarray(scale, f), (P, D)))
    offset_b = np.ascontiguousarray(
        np.broadcast_to(np.asarray(offset, f), (P, D)))
    in_maps = []
    for c in range(N_CORES):
        b, half = divmod(c, 2)
        sl = slice(half * SQ, (half + 1) * SQ)
        in_maps.append({
            "qT": np.ascontiguousarray(q16[b, sl].T),
            "qres": np.ascontiguousarray(q[b, sl]),
            "kT": np.ascontiguousarray(k16[b].T),
            "vT": np.ascontiguousarray(v16[b].T),
            "wqT": wqT, "wkT": wkT, "wvT": wvT, "wpT": wpT,
            "scale_b": scale_b, "offset_b": offset_b,
        })
    return in_maps


def kernel(q, k, v, Wq, Wk, Wv, Wp, scale, offset):
    global LAST_RESULT
    in_maps = make_in_maps(q, k, v, Wq, Wk, Wv, Wp, scale, offset)
    nc = _get_program()
    res = run_bass_kernel_spmd(nc, in_maps, list(range(N_CORES)))
    LAST_RESULT = res
    out = np.empty((B, S, D), np.float32)
    for c in range(N_CORES):
        b, half = divmod(c, 2)
        out[b, half * SQ:(half + 1) * SQ] = res.results[c]["out"]
    return out


# revision 9
# speedup vs baseline: 2.5484x; 1.2058x over previous
"""Multi-head attention + out-proj + residual + LayerNorm on 8 trn2 cores.

Sharding: (batch, seq-half) -> 8 shards, collective-free. Each core gets
transposed activations (host-prepped) plus shared (transposed) weights and
computes its full [1024, 1024] output block:

  phase V: V_all[Sk, H, dv]   = vT.T per-head proj  (kept in SBUF, +ones col)
  phase K: KT_all[H*dk, Sk]   -> staged to DRAM
  phase Q: QT_all[H*dk, Sq]   (kept in SBUF)
  attn  : per head: scoresT[Sk,Sq] = KT_h.T@QT_h -> exp(x/sqrt(D)) (ACT)
          OT[dv+1, Sq] += [V_h|1].T @ expT   (row 64 = softmax denom)
          OT[0:64] *= bcast(1/denom)  -> staged to DRAM (concat.T layout)
  final : out = LN(concatT.T @ WpT + q_res) * scale + offset
"""

import os
from contextlib import ExitStack

import numpy as np

import concourse.bass as bass
import concourse.tile as tile
from concourse import bacc, mybir
from concourse._compat import with_exitstack
from concourse.bass_utils import run_bass_kernel_spmd

B, S, D = 4, 2048, 1024
H, DK, DV = 16, 64, 64
F = H * DV            # 1024 flattened head dim (== H*DK)
N_CORES = 8
SQ = S // 2           # 1024 queries per core
SK = S                # 2048 keys per core
P = 128
KD = D // P           # 8 contraction chunks over d_model
NF = F // P           # 8 head-pair chunks
NSK = SK // P         # 16 key chunks
TEMP = float(np.sqrt(D))
EPS = 1e-9

F32 = mybir.dt.float32
BF16 = mybir.dt.bfloat16
F32R = mybir.dt.float32r

LAST_RESULT = None    # BassKernelResults of the most recent kernel() call


@with_exitstack
def _mha_kernel(ctx: ExitStack, tc: tile.TileContext, out_ap, ins):
    nc = tc.nc
    AF = mybir.ActivationFunctionType
    ALU = mybir.AluOpType

    g_const = ctx.enter_context(tc.tile_pool(name="gconst", bufs=1))
    dram = ctx.enter_context(tc.tile_pool(name="dramstage", bufs=1, space="DRAM"))

    kt_stage = dram.tile([NF, P, SK], BF16)   # KT_all, head-pair-chunk major
    ot_stage = dram.tile([NF, P, SQ], BF16)   # concat.T, head-pair-chunk major

    ones_sb = g_const.tile([P, 64], BF16)
    nc.vector.memset(ones_sb, 1.0)

    xq_r = ins["qT"].rearrange("(c p) s -> p c s", p=P)
    xk_r = ins["kT"].rearrange("(c p) s -> p c s", p=P)
    xv_r = ins["vT"].rearrange("(c p) s -> p c s", p=P)

    with (
        tc.tile_pool(name="resident", bufs=1) as resident,
        tc.tile_pool(name="wts", bufs=2) as wpool,
    ):
        # V_all with a ones column appended per head: [sk_part, sk, head, 65]
        v_sb = resident.tile([P, NSK, H, 65], BF16)
        qt_sb = resident.tile([P, NF, SQ], BF16)
        nc.vector.memset(v_sb[:, :, :, 64:65], 1.0)

        # ---------------- V projection: V_all[Sk, F] (natural layout) ------
        wv = wpool.tile([P, KD, F], BF16, tag="w")
        nc.sync.dma_start(wv, ins["wvT"].rearrange("(c p) f -> p c f", p=P))
        with (
            tc.tile_pool(name="xv", bufs=12) as xpool,
            tc.tile_pool(name="vps", bufs=4, space="PSUM") as vps,
        ):
            for sk in range(NSK):
                xts = []
                for kd in range(KD):
                    t = xpool.tile([P, P], BF16, tag="x", name="xt")
                    nc.sync.dma_start(t, xv_r[:, kd, sk * P:(sk + 1) * P])
                    xts.append(t)
                pss = [vps.tile([P, 512], F32, tag="ps", name="vp") for _ in range(2)]
                for kd in range(KD):
                    for n in range(2):
                        nc.tensor.matmul(
                            pss[n],
                            lhsT=xts[kd],
                            rhs=wv[:, kd, n * 512:(n + 1) * 512],
                            start=(kd == 0),
                            stop=(kd == KD - 1),
                        )
                for n in range(2):
                    nc.scalar.copy(
                        v_sb[:, sk, n * 8:(n + 1) * 8, 0:64],
                        pss[n].rearrange("p (h e) -> p h e", h=8),
                    )

        # ---------------- K projection: KT_all[F, Sk] -> DRAM --------------
        wk = wpool.tile([P, KD, F], BF16, tag="w")
        nc.sync.dma_start(wk, ins["wkT"].rearrange("(c p) f -> p c f", p=P))
        with (
            tc.tile_pool(name="xk", bufs=8) as xpool,
            tc.tile_pool(name="kout", bufs=4) as kout,
            tc.tile_pool(name="kps", bufs=4, space="PSUM") as kps,
        ):
            for n in range(SK // 512):  # 4 key blocks
                xts = []
                for kd in range(KD):
                    t = xpool.tile([P, 512], BF16, tag="x", name="xt")
                    nc.sync.dma_start(t, xk_r[:, kd, n * 512:(n + 1) * 512])
                    xts.append(t)
                for f in range(NF):
                    ps = kps.tile([P, 512], F32, tag="ps")
                    for kd in range(KD):
                        nc.tensor.matmul(
                            ps,
                            lhsT=wk[:, kd, f * P:(f + 1) * P],
                            rhs=xts[kd],
                            start=(kd == 0),
                            stop=(kd == KD - 1),
                        )
                    ko = kout.tile([P, 512], BF16, tag="ko")
                    nc.scalar.copy(ko, ps)
                    nc.sync.dma_start(kt_stage[f, :, n * 512:(n + 1) * 512], ko)

        # ---------------- Q projection: QT_all[F, Sq] -> SBUF --------------
        wq = wpool.tile([P, KD, F], BF16, tag="w")
        nc.sync.dma_start(wq, ins["wqT"].rearrange("(c p) f -> p c f", p=P))
        with (
            tc.tile_pool(name="xq", bufs=8) as xpool,
            tc.tile_pool(name="qps", bufs=4, space="PSUM") as qps,
        ):
            for n in range(SQ // 512):  # 2 query blocks
                xts = []
                for kd in range(KD):
                    t = xpool.tile([P, 512], BF16, tag="x", name="xt")
                    nc.sync.dma_start(t, xq_r[:, kd, n * 512:(n + 1) * 512])
                    xts.append(t)
                for f in range(NF):
                    ps = qps.tile([P, 512], F32, tag="ps")
                    for kd in range(KD):
                        nc.tensor.matmul(
                            ps,
                            lhsT=wq[:, kd, f * P:(f + 1) * P],
                            rhs=xts[kd],
                            start=(kd == 0),
                            stop=(kd == KD - 1),
                        )
                    nc.scalar.copy(qt_sb[:, f, n * 512:(n + 1) * 512], ps)

        # ---------------- attention, head by head --------------------------
        with (
            tc.tile_pool(name="ktc", bufs=2) as ktp,
            tc.tile_pool(name="expp", bufs=2) as expp,
            tc.tile_pool(name="rcp", bufs=2) as rcp,
            tc.tile_pool(name="bcs", bufs=2) as bcs,
            tc.tile_pool(name="oto", bufs=2) as oto,
            tc.tile_pool(name="scps", bufs=2, space="PSUM") as scps,
            tc.tile_pool(name="smps", bufs=4, space="PSUM") as smps,
        ):
            for c in range(NF):  # head-pair chunks
                ktc = ktp.tile([P, SK], BF16, tag="kt")
                nc.sync.dma_start(ktc, kt_stage[c])
                for hh in range(2):
                    h = 2 * c + hh
                    base = hh * 64
                    ot_ps = [
                        smps.tile([65, 512], F32, tag="sm", name="otp")
                        for _ in range(2)
                    ]
                    for sk in range(NSK):
                        sc = scps.tile([P, 2, 512], F32, tag="sc")
                        for sq in range(2):
                            nc.tensor.matmul(
                                sc[:, sq, :],
                                lhsT=ktc[base:base + 64, sk * P:(sk + 1) * P],
                                rhs=qt_sb[base:base + 64, c,
                                          sq * 512:(sq + 1) * 512],
                                start=True,
                                stop=True,
                            )
                        ex = expp.tile([P, 2, 512], BF16, tag="ex")
                        nc.scalar.activation(ex, sc, AF.Exp, scale=1.0 / TEMP)
                        for sq in range(2):
                            nc.tensor.matmul(
                                ot_ps[sq],
                                lhsT=v_sb[:, sk, h, :],
                                rhs=ex[:, sq, :],
                                start=(sk == 0),
                                stop=(sk == NSK - 1),
                            )
                    for sq in range(2):
                        # row 64 of ot_ps = softmax denominator
                        rc = rcp.tile([65, 512], F32, tag="rc")
                        nc.vector.reciprocal(rc[64:65, :], ot_ps[sq][64:65, :])
                        rcb = rcp.tile([65, 512], BF16, tag="rcb")
                        nc.vector.tensor_copy(rcb[64:65, :], rc[64:65, :])
                        bc_ps = smps.tile([64, 512], F32, tag="sm")
                        nc.tensor.matmul(
                            bc_ps,
                            lhsT=ones_sb[64:65, 0:64],
                            rhs=rcb[64:65, :],
                            start=True,
                            stop=True,
                        )
                        bc = bcs.tile([64, 512], F32, tag="bc")
                        nc.vector.tensor_copy(bc, bc_ps)
                        oo = oto.tile([64, 512], BF16, tag="oo")
                        nc.vector.tensor_mul(oo, ot_ps[sq][0:64, :], bc)
                        nc.sync.dma_start(
                            ot_stage[c, base:base + 64, sq * 512:(sq + 1) * 512],
                            oo,
                        )

    # ---------------- output projection + residual + layernorm -------------
    with (
        tc.tile_pool(name="wp", bufs=1) as wpp,
        tc.tile_pool(name="lnc", bufs=1) as lnc,
        tc.tile_pool(name="otf", bufs=12) as otf,
        tc.tile_pool(name="qres", bufs=2) as qrp,
        tc.tile_pool(name="lnw", bufs=2) as lnw,
        tc.tile_pool(name="stat", bufs=4) as stp,
        tc.tile_pool(name="fps", bufs=2, space="PSUM") as fps,
    ):
        wp = wpp.tile([P, NF, D], BF16)
        nc.sync.dma_start(wp, ins["wpT"].rearrange("(c p) f -> p c f", p=P))
        scale_sb = lnc.tile([P, 2, 512], F32)
        nc.sync.dma_start(scale_sb, ins["scale_b"].rearrange("p (a b) -> p a b", a=2))
        offset_sb = lnc.tile([P, 2, 512], F32)
        nc.sync.dma_start(offset_sb, ins["offset_b"].rearrange("p (a b) -> p a b", a=2))

        for sq in range(SQ // P):  # 8 query chunks of 128
            ots = []
            for f in range(NF):
                t = otf.tile([P, P], BF16, tag="ot", name="ott")
                nc.sync.dma_start(t, ot_stage[f, :, sq * P:(sq + 1) * P])
                ots.append(t)
            qr = qrp.tile([P, 2, 512], F32, tag="qr")
            nc.sync.dma_start(
                qr,
                ins["qres"][sq * P:(sq + 1) * P, :].rearrange(
                    "p (a b) -> p a b", a=2),
            )
            fp = fps.tile([P, 2, 512], F32, tag="fp")
            for d in range(2):
                for f in range(NF):
                    nc.tensor.matmul(
                        fp[:, d, :],
                        lhsT=ots[f],
                        rhs=wp[:, f, d * 512:(d + 1) * 512],
                        start=(f == 0),
                        stop=(f == NF - 1),
                    )
            x = lnw.tile([P, 2, 512], F32, tag="x")
            nc.vector.tensor_add(x, fp, qr)
            stats = stp.tile([P, 2, 6], F32, tag="st")
            for gsub in range(2):
                nc.vector.bn_stats(stats[:, gsub, :], x[:, gsub, :])
            mv = stp.tile([P, 2], F32, tag="mv")
            nc.vector.bn_aggr(mv, stats)
            # unbiased std + eps, then reciprocal
            stdt = stp.tile([P, 1], F32, tag="sd")
            nc.scalar.activation(stdt, mv[:, 1:2], AF.Sqrt,
                                 scale=float(D) / float(D - 1))
            nc.vector.tensor_scalar_add(stdt, stdt, EPS)
            rstd = stp.tile([P, 1], F32, tag="rs")
            nc.vector.reciprocal(rstd, stdt)
            xn = lnw.tile([P, 2, 512], F32, tag="xn")
            nc.vector.tensor_scalar(xn, x, mv[:, 0:1], rstd,
                                    ALU.subtract, ALU.mult)
            nc.vector.tensor_mul(xn, xn, scale_sb)
            nc.vector.tensor_add(xn, xn, offset_sb)
            nc.sync.dma_start(
                out_ap[sq * P:(sq + 1) * P, :],
                xn.rearrange("p a b -> p (a b)"),
            )


def build_program():
    nc = bacc.Bacc("TRN2", debug=False, target_bir_lowering=False)
    shapes = {
        "qT": ([D, SQ], BF16), "kT": ([D, SK], BF16), "vT": ([D, SK], BF16),
        "qres": ([SQ, D], F32),
        "wqT": ([D, F], BF16), "wkT": ([D, F], BF16), "wvT": ([D, F], BF16),
        "wpT": ([F, D], BF16),
        "scale_b": ([P, D], F32), "offset_b": ([P, D], F32),
    }
    ins = {k: nc.dram_tensor(k, shp, dt, kind="ExternalInput").ap()
           for k, (shp, dt) in shapes.items()}
    out = nc.dram_tensor("out", [SQ, D], F32, kind="ExternalOutput").ap()
    with tile.TileContext(nc) as tc:
        _mha_kernel(tc, out, ins)
    nc.compile()
    return nc


_PROGRAM = None


def _get_program():
    global _PROGRAM
    if _PROGRAM is None:
        _PROGRAM = build_program()
    return _PROGRAM


def make_in_maps(q, k, v, Wq, Wk, Wv, Wp, scale, offset):
    import ml_dtypes
    f = np.float32
    bf = ml_dtypes.bfloat16
    q = np.asarray(q, f)
    k16 = np.asarray(k, f).astype(bf)
    v16 = np.asarray(v, f).astype(bf)
    q16 = q.astype(bf)
    wqT = np.ascontiguousarray(
        np.asarray(Wq, f).transpose(2, 0, 1).reshape(D, F).astype(bf))
    wkT = np.ascontiguousarray(
        np.asarray(Wk, f).transpose(2, 0, 1).reshape(D, F).astype(bf))
    wvT = np.ascontiguousarray(
        np.asarray(Wv, f).transpose(2, 0, 1).reshape(D, F).astype(bf))
    wpT = np.ascontiguousarray(np.asarray(Wp, f).T.astype(bf))
    scale_b = np.ascontiguousarray(
        np.broadcast_to(np.asarray(scale, f), (P, D)))
    offset_b = np.ascontiguousarray(
        np.broadcast_to(np.asarray(offset, f), (P, D)))
    in_maps = []
    for c in range(N_CORES):
        b, half = divmod(c, 2)
        sl = slice(half * SQ, (half + 1) * SQ)
        in_maps.append({
            "qT": np.ascontiguousarray(q16[b, sl].T),
            "qres": np.ascontiguousarray(q[b, sl]),
            "kT": np.ascontiguousarray(k16[b].T),
            "vT": np.ascontiguousarray(v16[b].T),
            "wqT": wqT, "wkT": wkT, "wvT": wvT, "wpT": wpT,
            "scale_b": scale_b, "offset_b": offset_b,
        })
    return in_maps


def kernel(q, k, v, Wq, Wk, Wv, Wp, scale, offset):
    global LAST_RESULT
    in_maps = make_in_maps(q, k, v, Wq, Wk, Wv, Wp, scale, offset)
    nc = _get_program()
    res = run_bass_kernel_spmd(nc, in_maps, list(range(N_CORES)))
    LAST_RESULT = res
    out = np.empty((B, S, D), np.float32)
    for c in range(N_CORES):
        b, half = divmod(c, 2)
        out[b, half * SQ:(half + 1) * SQ] = res.results[c]["out"]
    return out


# revision 10
# speedup vs baseline: 3.1247x; 1.2261x over previous
"""Multi-head attention + out-proj + residual + LayerNorm on 8 trn2 cores.

Sharding: (batch, seq-half) -> 8 shards, collective-free. Each core gets
transposed activations (host-prepped) plus shared (transposed) weights and
computes its full [1024, 1024] output block:

  phase V: V_all[Sk, H, dv]   = vT.T per-head proj  (kept in SBUF, +ones col)
  phase K: KT_all[H*dk, Sk]   -> staged to DRAM
  phase Q: QT_all[H*dk, Sq]   (kept in SBUF)
  attn  : per head: scoresT[Sk,Sq] = KT_h.T@QT_h -> exp(x/sqrt(D)) (ACT)
          OT[dv+1, Sq] += [V_h|1].T @ expT   (row 64 = softmax denom)
          OT[0:64] *= bcast(1/denom)  -> staged to DRAM (concat.T layout)
  final : out = LN(concatT.T @ WpT + q_res) * scale + offset
"""

import os
from contextlib import ExitStack

import numpy as np

import concourse.bass as bass
import concourse.tile as tile
from concourse import bacc, mybir
from concourse._compat import with_exitstack
from concourse.bass_utils import run_bass_kernel_spmd

B, S, D = 4, 2048, 1024
H, DK, DV = 16, 64, 64
F = H * DV            # 1024 flattened head dim (== H*DK)
N_CORES = 8
SQ = S // 2           # 1024 queries per core
SK = S                # 2048 keys per core
P = 128
KD = D // P           # 8 contraction chunks over d_model
NF = F // P           # 8 head-pair chunks
NSK = SK // P         # 16 key chunks
TEMP = float(np.sqrt(D))
EPS = 1e-9

F32 = mybir.dt.float32
BF16 = mybir.dt.bfloat16
F32R = mybir.dt.float32r

LAST_RESULT = None    # BassKernelResults of the most recent kernel() call


@with_exitstack
def _mha_kernel(ctx: ExitStack, tc: tile.TileContext, out_ap, ins):
    nc = tc.nc
    AF = mybir.ActivationFunctionType
    ALU = mybir.AluOpType

    g_const = ctx.enter_context(tc.tile_pool(name="gconst", bufs=1))
    dram = ctx.enter_context(tc.tile_pool(name="dramstage", bufs=1, space="DRAM"))

    kt_stage = dram.tile([NF, P, SK], BF16)   # KT_all, head-pair-chunk major
    ot_stage = dram.tile([NF, P, SQ], BF16)   # concat.T, head-pair-chunk major

    ones_sb = g_const.tile([P, 64], BF16)
    nc.vector.memset(ones_sb, 1.0)

    xq_r = ins["qT"].rearrange("(c p) s -> p c s", p=P)
    xk_r = ins["kT"].rearrange("(c p) s -> p c s", p=P)
    xv_r = ins["vT"].rearrange("(c p) s -> p c s", p=P)

    with (
        tc.tile_pool(name="resident", bufs=1) as resident,
        tc.tile_pool(name="wts", bufs=2) as wpool,
    ):
        # V_all with a ones column appended per head: [sk_part, sk, head, 65]
        v_sb = resident.tile([P, NSK, H, 65], BF16)
        qt_sb = resident.tile([P, NF, SQ], BF16)
        nc.vector.memset(v_sb[:, :, :, 64:65], 1.0)

        # ---------------- V projection: V_all[Sk, F] (natural layout) ------
        wv = wpool.tile([P, KD, F], BF16, tag="w")
        nc.sync.dma_start(wv, ins["wvT"].rearrange("(c p) f -> p c f", p=P))
        with (
            tc.tile_pool(name="xv", bufs=12) as xpool,
            tc.tile_pool(name="vps", bufs=4, space="PSUM") as vps,
        ):
            for sk in range(NSK):
                xts = []
                for kd in range(KD):
                    t = xpool.tile([P, P], BF16, tag="x", name="xt")
                    nc.sync.dma_start(t, xv_r[:, kd, sk * P:(sk + 1) * P])
                    xts.append(t)
                pss = [vps.tile([P, 512], F32, tag="ps", name="vp") for _ in range(2)]
                for kd in range(KD):
                    for n in range(2):
                        nc.tensor.matmul(
                            pss[n],
                            lhsT=xts[kd],
                            rhs=wv[:, kd, n * 512:(n + 1) * 512],
                            start=(kd == 0),
                            stop=(kd == KD - 1),
                        )
                for n in range(2):
                    nc.scalar.copy(
                        v_sb[:, sk, n * 8:(n + 1) * 8, 0:64],
                        pss[n].rearrange("p (h e) -> p h e", h=8),
                    )

        # ---------------- K projection: KT_all[F, Sk] -> DRAM --------------
        wk = wpool.tile([P, KD, F], BF16, tag="w")
        nc.sync.dma_start(wk, ins["wkT"].rearrange("(c p) f -> p c f", p=P))
        with (
            tc.tile_pool(name="xk", bufs=8) as xpool,
            tc.tile_pool(name="kout", bufs=4) as kout,
            tc.tile_pool(name="kps", bufs=4, space="PSUM") as kps,
        ):
            for n in range(SK // 512):  # 4 key blocks
                xts = []
                for kd in range(KD):
                    t = xpool.tile([P, 512], BF16, tag="x", name="xt")
                    nc.sync.dma_start(t, xk_r[:, kd, n * 512:(n + 1) * 512])
                    xts.append(t)
                for f in range(NF):
                    ps = kps.tile([P, 512], F32, tag="ps")
                    for kd in range(KD):
                        nc.tensor.matmul(
                            ps,
                            lhsT=wk[:, kd, f * P:(f + 1) * P],
                            rhs=xts[kd],
                            start=(kd == 0),
                            stop=(kd == KD - 1),
                        )
                    ko = kout.tile([P, 512], BF16, tag="ko")
                    nc.scalar.copy(ko, ps)
                    nc.sync.dma_start(kt_stage[f, :, n * 512:(n + 1) * 512], ko)

        # ---------------- Q projection: QT_all[F, Sq] -> SBUF --------------
        wq = wpool.tile([P, KD, F], BF16, tag="w")
        nc.sync.dma_start(wq, ins["wqT"].rearrange("(c p) f -> p c f", p=P))
        with (
            tc.tile_pool(name="xq", bufs=8) as xpool,
            tc.tile_pool(name="qps", bufs=4, space="PSUM") as qps,
        ):
            for n in range(SQ // 512):  # 2 query blocks
                xts = []
                for kd in range(KD):
                    t = xpool.tile([P, 512], BF16, tag="x", name="xt")
                    nc.sync.dma_start(t, xq_r[:, kd, n * 512:(n + 1) * 512])
                    xts.append(t)
                for f in range(NF):
                    ps = qps.tile([P, 512], F32, tag="ps")
                    for kd in range(KD):
                        nc.tensor.matmul(
                            ps,
                            lhsT=wq[:, kd, f * P:(f + 1) * P],
                            rhs=xts[kd],
                            start=(kd == 0),
                            stop=(kd == KD - 1),
                        )
                    nc.scalar.copy(qt_sb[:, f, n * 512:(n + 1) * 512], ps)

        # ---------------- attention, head by head --------------------------
        with (
            tc.tile_pool(name="ktc", bufs=2) as ktp,
            tc.tile_pool(name="expp", bufs=2) as expp,
            tc.tile_pool(name="rcp", bufs=2) as rcp,
            tc.tile_pool(name="bcs", bufs=2) as bcs,
            tc.tile_pool(name="oto", bufs=2) as oto,
            tc.tile_pool(name="scps", bufs=2, space="PSUM") as scps,
            tc.tile_pool(name="smps", bufs=4, space="PSUM") as smps,
        ):
            for c in range(NF):  # head-pair chunks
                ktc = ktp.tile([P, SK], BF16, tag="kt")
                nc.sync.dma_start(ktc, kt_stage[c])
                for sq in range(2):
                    # both heads of the pair accumulate together; their
                    # score matmuls occupy disjoint PE row groups (rows
                    # 0-63 / 64-127) and run concurrently.
                    ot_ps = [
                        smps.tile([65, 512], F32, tag="sm", name="otp")
                        for _ in range(2)
                    ]
                    for sk in range(NSK):
                        sc = scps.tile([P, 2, 512], F32, tag="sc")
                        for hh in range(2):
                            base = hh * 64
                            nc.tensor.matmul(
                                sc[:, hh, :],
                                lhsT=ktc[base:base + 64, sk * P:(sk + 1) * P],
                                rhs=qt_sb[base:base + 64, c,
                                          sq * 512:(sq + 1) * 512],
                                start=True,
                                stop=True,
                            )
                        ex = expp.tile([P, 2, 512], BF16, tag="ex")
                        nc.scalar.activation(ex, sc, AF.Exp, scale=1.0 / TEMP)
                        for hh in range(2):
                            nc.tensor.matmul(
                                ot_ps[hh],
                                lhsT=v_sb[:, sk, 2 * c + hh, :],
                                rhs=ex[:, hh, :],
                                start=(sk == 0),
                                stop=(sk == NSK - 1),
                            )
                    for hh in range(2):
                        base = hh * 64
                        # row 64 of ot_ps = softmax denominator
                        rc = rcp.tile([65, 512], F32, tag="rc")
                        nc.vector.reciprocal(rc[64:65, :], ot_ps[hh][64:65, :])
                        rcb = rcp.tile([65, 512], BF16, tag="rcb")
                        nc.vector.tensor_copy(rcb[64:65, :], rc[64:65, :])
                        bc_ps = smps.tile([64, 512], F32, tag="sm")
                        nc.tensor.matmul(
                            bc_ps,
                            lhsT=ones_sb[64:65, 0:64],
                            rhs=rcb[64:65, :],
                            start=True,
                            stop=True,
                        )
                        bc = bcs.tile([64, 512], F32, tag="bc")
                        nc.vector.tensor_copy(bc, bc_ps)
                        oo = oto.tile([64, 512], BF16, tag="oo")
                        nc.vector.tensor_mul(oo, ot_ps[hh][0:64, :], bc)
                        nc.sync.dma_start(
                            ot_stage[c, base:base + 64, sq * 512:(sq + 1) * 512],
                            oo,
                        )

    # ---------------- output projection + residual + layernorm -------------
    with (
        tc.tile_pool(name="wp", bufs=1) as wpp,
        tc.tile_pool(name="lnc", bufs=1) as lnc,
        tc.tile_pool(name="otf", bufs=12) as otf,
        tc.tile_pool(name="qres", bufs=2) as qrp,
        tc.tile_pool(name="lnw", bufs=2) as lnw,
        tc.tile_pool(name="stat", bufs=4) as stp,
        tc.tile_pool(name="fps", bufs=2, space="PSUM") as fps,
    ):
        wp = wpp.tile([P, NF, D], BF16)
        nc.sync.dma_start(wp, ins["wpT"].rearrange("(c p) f -> p c f", p=P))
        scale_sb = lnc.tile([P, 2, 512], F32)
        nc.sync.dma_start(scale_sb, ins["scale_b"].rearrange("p (a b) -> p a b", a=2))
        offset_sb = lnc.tile([P, 2, 512], F32)
        nc.sync.dma_start(offset_sb, ins["offset_b"].rearrange("p (a b) -> p a b", a=2))

        for sq in range(SQ // P):  # 8 query chunks of 128
            ots = []
            for f in range(NF):
                t = otf.tile([P, P], BF16, tag="ot", name="ott")
                nc.sync.dma_start(t, ot_stage[f, :, sq * P:(sq + 1) * P])
                ots.append(t)
            qr = qrp.tile([P, 2, 512], F32, tag="qr")
            nc.sync.dma_start(
                qr,
                ins["qres"][sq * P:(sq + 1) * P, :].rearrange(
                    "p (a b) -> p a b", a=2),
            )
            fp = fps.tile([P, 2, 512], F32, tag="fp")
            for d in range(2):
                for f in range(NF):
                    nc.tensor.matmul(
                        fp[:, d, :],
                        lhsT=ots[f],
                        rhs=wp[:, f, d * 512:(d + 1) * 512],
                        start=(f == 0),
                        stop=(f == NF - 1),
                    )
            x = lnw.tile([P, 2, 512], F32, tag="x")
            nc.vector.tensor_add(x, fp, qr)
            stats = stp.tile([P, 2, 6], F32, tag="st")
            for gsub in range(2):
                nc.vector.bn_stats(stats[:, gsub, :], x[:, gsub, :])
            mv = stp.tile([P, 2], F32, tag="mv")
            nc.vector.bn_aggr(mv, stats)
            # unbiased std + eps, then reciprocal
            stdt = stp.tile([P, 1], F32, tag="sd")
            nc.scalar.activation(stdt, mv[:, 1:2], AF.Sqrt,
                                 scale=float(D) / float(D - 1))
            nc.vector.tensor_scalar_add(stdt, stdt, EPS)
            rstd = stp.tile([P, 1], F32, tag="rs")
            nc.vector.reciprocal(rstd, stdt)
            xn = lnw.tile([P, 2, 512], F32, tag="xn")
            nc.vector.tensor_scalar(xn, x, mv[:, 0:1], rstd,
                                    ALU.subtract, ALU.mult)
            nc.vector.tensor_mul(xn, xn, scale_sb)
            nc.vector.tensor_add(xn, xn, offset_sb)
            nc.sync.dma_start(
                out_ap[sq * P:(sq + 1) * P, :],
                xn.rearrange("p a b -> p (a b)"),
            )


def build_program():
    nc = bacc.Bacc("TRN2", debug=False, target_bir_lowering=False)
    shapes = {
        "qT": ([D, SQ], BF16), "kT": ([D, SK], BF16), "vT": ([D, SK], BF16),
        "qres": ([SQ, D], F32),
        "wqT": ([D, F], BF16), "wkT": ([D, F], BF16), "wvT": ([D, F], BF16),
        "wpT": ([F, D], BF16),
        "scale_b": ([P, D], F32), "offset_b": ([P, D], F32),
    }
    ins = {k: nc.dram_tensor(k, shp, dt, kind="ExternalInput").ap()
           for k, (shp, dt) in shapes.items()}
    out = nc.dram_tensor("out", [SQ, D], F32, kind="ExternalOutput").ap()
    with tile.TileContext(nc) as tc:
        _mha_kernel(tc, out, ins)
    nc.compile()
    return nc


_PROGRAM = None


def _get_program():
    global _PROGRAM
    if _PROGRAM is None:
        _PROGRAM = build_program()
    return _PROGRAM


def make_in_maps(q, k, v, Wq, Wk, Wv, Wp, scale, offset):
    import ml_dtypes
    f = np.float32
    bf = ml_dtypes.bfloat16
    q = np.asarray(q, f)
    k16 = np.asarray(k, f).astype(bf)
    v16 = np.asarray(v, f).astype(bf)
    q16 = q.astype(bf)
    wqT = np.ascontiguousarray(
        np.asarray(Wq, f).transpose(2, 0, 1).reshape(D, F).astype(bf))
    wkT = np.ascontiguousarray(
        np.asarray(Wk, f).transpose(2, 0, 1).reshape(D, F).astype(bf))
    wvT = np.ascontiguousarray(
        np.asarray(Wv, f).transpose(2, 0, 1).reshape(D, F).astype(bf))
    wpT = np.ascontiguousarray(np.asarray(Wp, f).T.astype(bf))
    scale_b = np.ascontiguousarray(
        np.broadcast_to(np.asarray(scale, f), (P, D)))
    offset_b = np.ascontiguousarray(
        np.broadcast_to(np.asarray(offset, f), (P, D)))
    in_maps = []
    for c in range(N_CORES):
        b, half = divmod(c, 2)
        sl = slice(half * SQ, (half + 1) * SQ)
        in_maps.append({
            "qT": np.ascontiguousarray(q16[b, sl].T),
            "qres": np.ascontiguousarray(q[b, sl]),
            "kT": np.ascontiguousarray(k16[b].T),
            "vT": np.ascontiguousarray(v16[b].T),
            "wqT": wqT, "wkT": wkT, "wvT": wvT, "wpT": wpT,
            "scale_b": scale_b, "offset_b": offset_b,
        })
    return in_maps


def kernel(q, k, v, Wq, Wk, Wv, Wp, scale, offset):
    global LAST_RESULT
    in_maps = make_in_maps(q, k, v, Wq, Wk, Wv, Wp, scale, offset)
    nc = _get_program()
    res = run_bass_kernel_spmd(nc, in_maps, list(range(N_CORES)))
    LAST_RESULT = res
    out = np.empty((B, S, D), np.float32)
    for c in range(N_CORES):
        b, half = divmod(c, 2)
        out[b, half * SQ:(half + 1) * SQ] = res.results[c]["out"]
    return out


# revision 11
# speedup vs baseline: 3.1289x; 1.0013x over previous
"""Multi-head attention + out-proj + residual + LayerNorm on 8 trn2 cores.

Sharding: (batch, seq-half) -> 8 shards, collective-free. Each core gets
transposed activations (host-prepped) plus shared (transposed) weights and
computes its full [1024, 1024] output block:

  phase V: V_all[Sk, H, dv]   = vT.T per-head proj  (kept in SBUF, +ones col)
  phase K: KT_all[H*dk, Sk]   -> staged to DRAM
  phase Q: QT_all[H*dk, Sq]   (kept in SBUF)
  attn  : per head: scoresT[Sk,Sq] = KT_h.T@QT_h -> exp(x/sqrt(D)) (ACT)
          OT[dv+1, Sq] += [V_h|1].T @ expT   (row 64 = softmax denom)
          OT[0:64] *= bcast(1/denom)  -> staged to DRAM (concat.T layout)
  final : out = LN(concatT.T @ WpT + q_res) * scale + offset
"""

import os
from contextlib import ExitStack

import numpy as np

import concourse.bass as bass
import concourse.tile as tile
from concourse import bacc, mybir
from concourse._compat import with_exitstack
from concourse.bass_utils import run_bass_kernel_spmd

B, S, D = 4, 2048, 1024
H, DK, DV = 16, 64, 64
F = H * DV            # 1024 flattened head dim (== H*DK)
N_CORES = 8
SQ = S // 2           # 1024 queries per core
SK = S                # 2048 keys per core
P = 128
KD = D // P           # 8 contraction chunks over d_model
NF = F // P           # 8 head-pair chunks
NSK = SK // P         # 16 key chunks
TEMP = float(np.sqrt(D))
EPS = 1e-9

F32 = mybir.dt.float32
BF16 = mybir.dt.bfloat16
F32R = mybir.dt.float32r

LAST_RESULT = None    # BassKernelResults of the most recent kernel() call


@with_exitstack
def _mha_kernel(ctx: ExitStack, tc: tile.TileContext, out_ap, ins):
    nc = tc.nc
    AF = mybir.ActivationFunctionType
    ALU = mybir.AluOpType

    g_const = ctx.enter_context(tc.tile_pool(name="gconst", bufs=1))
    dram = ctx.enter_context(tc.tile_pool(name="dramstage", bufs=1, space="DRAM"))

    kt_stage = dram.tile([NF, P, SK], BF16)   # KT_all, head-pair-chunk major
    ot_stage = dram.tile([NF, P, SQ], BF16)   # concat.T, head-pair-chunk major

    ones_sb = g_const.tile([P, 64], BF16)
    nc.vector.memset(ones_sb, 1.0)

    xq_r = ins["qT"].rearrange("(c p) s -> p c s", p=P)
    xk_r = ins["kT"].rearrange("(c p) s -> p c s", p=P)
    xv_r = ins["vT"].rearrange("(c p) s -> p c s", p=P)

    with (
        tc.tile_pool(name="resident", bufs=1) as resident,
        tc.tile_pool(name="wts", bufs=2) as wpool,
    ):
        # V_all with a ones column appended per head: [sk_part, sk, head, 65]
        v_sb = resident.tile([P, NSK, H, 65], BF16)
        qt_sb = resident.tile([P, NF, SQ], BF16)
        nc.vector.memset(v_sb[:, :, :, 64:65], 1.0)

        # ---------------- V projection: V_all[Sk, F] (natural layout) ------
        # F-half outer: heads 0-7 land in v_sb first so attention can begin
        # before the projections finish.
        wv = wpool.tile([P, KD, F], BF16, tag="w")
        nc.sync.dma_start(wv, ins["wvT"].rearrange("(c p) f -> p c f", p=P))
        with (
            tc.tile_pool(name="xv", bufs=1) as xpool,
            tc.tile_pool(name="vps", bufs=4, space="PSUM") as vps,
        ):
            xv = xpool.tile([P, KD, SK], BF16)
            nc.sync.dma_start(xv, xv_r)
            for n in range(2):
                for sk in range(NSK):
                    ps = vps.tile([P, 512], F32, tag="ps", name="vp")
                    for kd in range(KD):
                        nc.tensor.matmul(
                            ps,
                            lhsT=xv[:, kd, sk * P:(sk + 1) * P],
                            rhs=wv[:, kd, n * 512:(n + 1) * 512],
                            start=(kd == 0),
                            stop=(kd == KD - 1),
                        )
                    nc.vector.tensor_copy(
                        v_sb[:, sk, n * 8:(n + 1) * 8, 0:64],
                        ps.rearrange("p (h e) -> p h e", h=8),
                    )

        # ---------------- K projection: KT_all[F, Sk] -> DRAM --------------
        wk = wpool.tile([P, KD, F], BF16, tag="w")
        nc.sync.dma_start(wk, ins["wkT"].rearrange("(c p) f -> p c f", p=P))
        with (
            tc.tile_pool(name="xk", bufs=1) as xpool,
            tc.tile_pool(name="kout", bufs=4) as kout,
            tc.tile_pool(name="kps", bufs=4, space="PSUM") as kps,
        ):
            xk = xpool.tile([P, KD, SK], BF16)
            nc.sync.dma_start(xk, xk_r)
            for f in range(NF):  # chunk-major so kt_stage[0] is ready first
                for n in range(SK // 512):
                    ps = kps.tile([P, 512], F32, tag="ps")
                    for kd in range(KD):
                        nc.tensor.matmul(
                            ps,
                            lhsT=wk[:, kd, f * P:(f + 1) * P],
                            rhs=xk[:, kd, n * 512:(n + 1) * 512],
                            start=(kd == 0),
                            stop=(kd == KD - 1),
                        )
                    ko = kout.tile([P, 512], BF16, tag="ko")
                    nc.vector.tensor_copy(ko, ps)
                    nc.sync.dma_start(kt_stage[f, :, n * 512:(n + 1) * 512], ko)

        # ---------------- Q projection: QT_all[F, Sq] -> SBUF --------------
        wq = wpool.tile([P, KD, F], BF16, tag="w")
        nc.sync.dma_start(wq, ins["wqT"].rearrange("(c p) f -> p c f", p=P))
        with (
            tc.tile_pool(name="xq", bufs=1) as xpool,
            tc.tile_pool(name="qps", bufs=4, space="PSUM") as qps,
        ):
            xq = xpool.tile([P, KD, SQ], BF16)
            nc.sync.dma_start(xq, xq_r)
            for f in range(NF):  # chunk-major so qt_sb[:, 0] is ready first
                for n in range(SQ // 512):
                    ps = qps.tile([P, 512], F32, tag="ps")
                    for kd in range(KD):
                        nc.tensor.matmul(
                            ps,
                            lhsT=wq[:, kd, f * P:(f + 1) * P],
                            rhs=xq[:, kd, n * 512:(n + 1) * 512],
                            start=(kd == 0),
                            stop=(kd == KD - 1),
                        )
                    nc.vector.tensor_copy(qt_sb[:, f, n * 512:(n + 1) * 512], ps)

        # ---------------- attention, head by head --------------------------
        with (
            tc.tile_pool(name="ktc", bufs=2) as ktp,
            tc.tile_pool(name="expp", bufs=2) as expp,
            tc.tile_pool(name="rcp", bufs=2) as rcp,
            tc.tile_pool(name="bcs", bufs=2) as bcs,
            tc.tile_pool(name="oto", bufs=2) as oto,
            tc.tile_pool(name="scps", bufs=2, space="PSUM") as scps,
            tc.tile_pool(name="smps", bufs=4, space="PSUM") as smps,
        ):
            for c in range(NF):  # head-pair chunks
                ktc = ktp.tile([P, SK], BF16, tag="kt")
                nc.sync.dma_start(ktc, kt_stage[c])
                for sq in range(2):
                    # both heads of the pair accumulate together; their
                    # score matmuls occupy disjoint PE row groups (rows
                    # 0-63 / 64-127) and run concurrently.
                    ot_ps = [
                        smps.tile([65, 512], F32, tag="sm", name="otp")
                        for _ in range(2)
                    ]
                    for sk in range(NSK):
                        sc = scps.tile([P, 2, 512], F32, tag="sc")
                        for hh in range(2):
                            base = hh * 64
                            nc.tensor.matmul(
                                sc[:, hh, :],
                                lhsT=ktc[base:base + 64, sk * P:(sk + 1) * P],
                                rhs=qt_sb[base:base + 64, c,
                                          sq * 512:(sq + 1) * 512],
                                start=True,
                                stop=True,
                            )
                        ex = expp.tile([P, 2, 512], BF16, tag="ex")
                        nc.scalar.activation(ex, sc, AF.Exp, scale=1.0 / TEMP)
                        for hh in range(2):
                            nc.tensor.matmul(
                                ot_ps[hh],
                                lhsT=v_sb[:, sk, 2 * c + hh, :],
                                rhs=ex[:, hh, :],
                                start=(sk == 0),
                                stop=(sk == NSK - 1),
                            )
                    for hh in range(2):
                        base = hh * 64
                        # row 64 of ot_ps = softmax denominator
                        rc = rcp.tile([65, 512], F32, tag="rc")
                        nc.vector.reciprocal(rc[64:65, :], ot_ps[hh][64:65, :])
                        rcb = rcp.tile([65, 512], BF16, tag="rcb")
                        nc.vector.tensor_copy(rcb[64:65, :], rc[64:65, :])
                        bc_ps = smps.tile([64, 512], F32, tag="sm")
                        nc.tensor.matmul(
                            bc_ps,
                            lhsT=ones_sb[64:65, 0:64],
                            rhs=rcb[64:65, :],
                            start=True,
                            stop=True,
                        )
                        bc = bcs.tile([64, 512], F32, tag="bc")
                        nc.vector.tensor_copy(bc, bc_ps)
                        oo = oto.tile([64, 512], BF16, tag="oo")
                        nc.vector.tensor_mul(oo, ot_ps[hh][0:64, :], bc)
                        nc.sync.dma_start(
                            ot_stage[c, base:base + 64, sq * 512:(sq + 1) * 512],
                            oo,
                        )

    # ---------------- output projection + residual + layernorm -------------
    with (
        tc.tile_pool(name="wp", bufs=1) as wpp,
        tc.tile_pool(name="lnc", bufs=1) as lnc,
        tc.tile_pool(name="otf", bufs=12) as otf,
        tc.tile_pool(name="qres", bufs=3) as qrp,
        tc.tile_pool(name="lnw", bufs=4) as lnw,
        tc.tile_pool(name="stat", bufs=8) as stp,
        tc.tile_pool(name="fps", bufs=2, space="PSUM") as fps,
    ):
        wp = wpp.tile([P, NF, D], BF16)
        nc.sync.dma_start(wp, ins["wpT"].rearrange("(c p) f -> p c f", p=P))
        scale_sb = lnc.tile([P, 2, 512], F32)
        nc.sync.dma_start(scale_sb, ins["scale_b"].rearrange("p (a b) -> p a b", a=2))
        offset_sb = lnc.tile([P, 2, 512], F32)
        nc.sync.dma_start(offset_sb, ins["offset_b"].rearrange("p (a b) -> p a b", a=2))

        for sq in range(SQ // P):  # 8 query chunks of 128
            ots = []
            for f in range(NF):
                t = otf.tile([P, P], BF16, tag="ot", name="ott")
                nc.sync.dma_start(t, ot_stage[f, :, sq * P:(sq + 1) * P])
                ots.append(t)
            qr = qrp.tile([P, 2, 512], F32, tag="qr")
            nc.sync.dma_start(
                qr,
                ins["qres"][sq * P:(sq + 1) * P, :].rearrange(
                    "p (a b) -> p a b", a=2),
            )
            fp = fps.tile([P, 2, 512], F32, tag="fp")
            for d in range(2):
                for f in range(NF):
                    nc.tensor.matmul(
                        fp[:, d, :],
                        lhsT=ots[f],
                        rhs=wp[:, f, d * 512:(d + 1) * 512],
                        start=(f == 0),
                        stop=(f == NF - 1),
                    )
            x = lnw.tile([P, 2, 512], F32, tag="x")
            nc.vector.tensor_add(x, fp, qr)
            stats = stp.tile([P, 2, 6], F32, tag="st")
            for gsub in range(2):
                nc.vector.bn_stats(stats[:, gsub, :], x[:, gsub, :])
            mv = stp.tile([P, 2], F32, tag="mv")
            nc.vector.bn_aggr(mv, stats)
            # unbiased std + eps, then reciprocal
            stdt = stp.tile([P, 1], F32, tag="sd")
            nc.scalar.activation(stdt, mv[:, 1:2], AF.Sqrt,
                                 scale=float(D) / float(D - 1))
            nc.vector.tensor_scalar_add(stdt, stdt, EPS)
            rstd = stp.tile([P, 1], F32, tag="rs")
            nc.vector.reciprocal(rstd, stdt)
            xn = lnw.tile([P, 2, 512], F32, tag="xn")
            nc.vector.tensor_scalar(xn, x, mv[:, 0:1], rstd,
                                    ALU.subtract, ALU.mult)
            nc.gpsimd.tensor_mul(xn, xn, scale_sb)
            nc.gpsimd.tensor_add(xn, xn, offset_sb)
            nc.sync.dma_start(
                out_ap[sq * P:(sq + 1) * P, :],
                xn.rearrange("p a b -> p (a b)"),
            )


def build_program():
    nc = bacc.Bacc("TRN2", debug=False, target_bir_lowering=False)
    shapes = {
        "qT": ([D, SQ], BF16), "kT": ([D, SK], BF16), "vT": ([D, SK], BF16),
        "qres": ([SQ, D], F32),
        "wqT": ([D, F], BF16), "wkT": ([D, F], BF16), "wvT": ([D, F], BF16),
        "wpT": ([F, D], BF16),
        "scale_b": ([P, D], F32), "offset_b": ([P, D], F32),
    }
    ins = {k: nc.dram_tensor(k, shp, dt, kind="ExternalInput").ap()
           for k, (shp, dt) in shapes.items()}
    out = nc.dram_tensor("out", [SQ, D], F32, kind="ExternalOutput").ap()
    with tile.TileContext(nc) as tc:
        _mha_kernel(tc, out, ins)
    nc.compile()
    return nc


_PROGRAM = None


def _get_program():
    global _PROGRAM
    if _PROGRAM is None:
        _PROGRAM = build_program()
    return _PROGRAM


def make_in_maps(q, k, v, Wq, Wk, Wv, Wp, scale, offset):
    import ml_dtypes
    f = np.float32
    bf = ml_dtypes.bfloat16
    q = np.asarray(q, f)
    k16 = np.asarray(k, f).astype(bf)
    v16 = np.asarray(v, f).astype(bf)
    q16 = q.astype(bf)
    wqT = np.ascontiguousarray(
        np.asarray(Wq, f).transpose(2, 0, 1).reshape(D, F).astype(bf))
    wkT = np.ascontiguousarray(
        np.asarray(Wk, f).transpose(2, 0, 1).reshape(D, F).astype(bf))
    wvT = np.ascontiguousarray(
        np.asarray(Wv, f).transpose(2, 0, 1).reshape(D, F).astype(bf))
    wpT = np.ascontiguousarray(np.asarray(Wp, f).T.astype(bf))
    scale_b = np.ascontiguousarray(
        np.broadcast_to(np.asarray(scale, f), (P, D)))
    offset_b = np.ascontiguousarray(
        np.broadcast_to(np.asarray(offset, f), (P, D)))
    in_maps = []
    for c in range(N_CORES):
        b, half = divmod(c, 2)
        sl = slice(half * SQ, (half + 1) * SQ)
        in_maps.append({
            "qT": np.ascontiguousarray(q16[b, sl].T),
            "qres": np.ascontiguousarray(q[b, sl]),
            "kT": np.ascontiguousarray(k16[b].T),
            "vT": np.ascontiguousarray(v16[b].T),
            "wqT": wqT, "wkT": wkT, "wvT": wvT, "wpT": wpT,
            "scale_b": scale_b, "offset_b": offset_b,
        })
    return in_maps


def kernel(q, k, v, Wq, Wk, Wv, Wp, scale, offset):
    global LAST_RESULT
    in_maps = make_in_maps(q, k, v, Wq, Wk, Wv, Wp, scale, offset)
    nc = _get_program()
    res = run_bass_kernel_spmd(nc, in_maps, list(range(N_CORES)))
    LAST_RESULT = res
    out = np.empty((B, S, D), np.float32)
    for c in range(N_CORES):
        b, half = divmod(c, 2)
        out[b, half * SQ:(half + 1) * SQ] = res.results[c]["out"]
    return out


# revision 13
# speedup vs baseline: 3.1577x; 1.0092x over previous
"""Multi-head attention + out-proj + residual + LayerNorm on 8 trn2 cores.

Sharding: (batch, seq-half) -> 8 shards, collective-free. Each core gets
transposed activations (host-prepped) plus shared (transposed) weights and
computes its full [1024, 1024] output block:

  phase V: V_all[Sk, H, dv]   = vT.T per-head proj  (kept in SBUF, +ones col)
  phase K: KT_all[H*dk, Sk]   -> staged to DRAM
  phase Q: QT_all[H*dk, Sq]   (kept in SBUF)
  attn  : per head: scoresT[Sk,Sq] = KT_h.T@QT_h -> exp(x/sqrt(D)) (ACT)
          OT[dv+1, Sq] += [V_h|1].T @ expT   (row 64 = softmax denom)
          OT[0:64] *= bcast(1/denom)  -> staged to DRAM (concat.T layout)
  final : out = LN(concatT.T @ WpT + q_res) * scale + offset
"""

import os
from contextlib import ExitStack

import numpy as np

import concourse.bass as bass
import concourse.tile as tile
from concourse import bacc, mybir
from concourse._compat import with_exitstack
from concourse.bass_utils import run_bass_kernel_spmd

B, S, D = 4, 2048, 1024
H, DK, DV = 16, 64, 64
F = H * DV            # 1024 flattened head dim (== H*DK)
N_CORES = 8
SQ = S // 2           # 1024 queries per core
SK = S                # 2048 keys per core
P = 128
KD = D // P           # 8 contraction chunks over d_model
NF = F // P           # 8 head-pair chunks
NSK = SK // P         # 16 key chunks
TEMP = float(np.sqrt(D))
EPS = 1e-9

F32 = mybir.dt.float32
BF16 = mybir.dt.bfloat16
F32R = mybir.dt.float32r

LAST_RESULT = None    # BassKernelResults of the most recent kernel() call


@with_exitstack
def _mha_kernel(ctx: ExitStack, tc: tile.TileContext, out_ap, ins):
    nc = tc.nc
    AF = mybir.ActivationFunctionType
    ALU = mybir.AluOpType

    g_const = ctx.enter_context(tc.tile_pool(name="gconst", bufs=1))
    dram = ctx.enter_context(tc.tile_pool(name="dramstage", bufs=1, space="DRAM"))

    kt_stage = dram.tile([NF, P, SK], BF16)   # KT_all, head-pair-chunk major
    ot_stage = dram.tile([NF, P, SQ], BF16)   # concat.T, head-pair-chunk major

    ones_sb = g_const.tile([P, 64], BF16)
    nc.vector.memset(ones_sb, 1.0)

    xq_r = ins["qT"].rearrange("(c p) s -> p c s", p=P)
    xk_r = ins["kT"].rearrange("(c p) s -> p c s", p=P)
    xv_r = ins["vT"].rearrange("(c p) s -> p c s", p=P)

    with (
        tc.tile_pool(name="resident", bufs=1) as resident,
        tc.tile_pool(name="wts", bufs=2) as wpool,
    ):
        # V_all with a ones column appended per head: [sk_part, sk, head, 65]
        v_sb = resident.tile([P, NSK, H, 65], BF16)
        qt_sb = resident.tile([P, NF, SQ], BF16)
        nc.vector.memset(v_sb[:, :, :, 64:65], 1.0)

        # ---------------- V projection: V_all[Sk, F] (natural layout) ------
        # F-half outer: heads 0-7 land in v_sb first so attention can begin
        # before the projections finish.
        wv = wpool.tile([P, KD, F], BF16, tag="w")
        nc.sync.dma_start(wv, ins["wvT"].rearrange("(c p) f -> p c f", p=P))
        with (
            tc.tile_pool(name="xv", bufs=1) as xpool,
            tc.tile_pool(name="vps", bufs=4, space="PSUM") as vps,
        ):
            xv = xpool.tile([P, KD, SK], BF16)
            nc.sync.dma_start(xv, xv_r)
            for n in range(2):
                for sk in range(NSK):
                    ps = vps.tile([P, 512], F32, tag="ps", name="vp")
                    for kd in range(KD):
                        nc.tensor.matmul(
                            ps,
                            lhsT=xv[:, kd, sk * P:(sk + 1) * P],
                            rhs=wv[:, kd, n * 512:(n + 1) * 512],
                            start=(kd == 0),
                            stop=(kd == KD - 1),
                        )
                    nc.vector.tensor_copy(
                        v_sb[:, sk, n * 8:(n + 1) * 8, 0:64],
                        ps.rearrange("p (h e) -> p h e", h=8),
                    )

        # ---------------- K projection: KT_all[F, Sk] -> DRAM --------------
        wk = wpool.tile([P, KD, F], BF16, tag="w")
        nc.sync.dma_start(wk, ins["wkT"].rearrange("(c p) f -> p c f", p=P))
        with (
            tc.tile_pool(name="xk", bufs=1) as xpool,
            tc.tile_pool(name="kout", bufs=4) as kout,
            tc.tile_pool(name="kps", bufs=4, space="PSUM") as kps,
        ):
            xk = xpool.tile([P, KD, SK], BF16)
            nc.sync.dma_start(xk, xk_r)
            for f in range(NF):  # chunk-major so kt_stage[0] is ready first
                for n in range(SK // 512):
                    ps = kps.tile([P, 512], F32, tag="ps")
                    for kd in range(KD):
                        nc.tensor.matmul(
                            ps,
                            lhsT=wk[:, kd, f * P:(f + 1) * P],
                            rhs=xk[:, kd, n * 512:(n + 1) * 512],
                            start=(kd == 0),
                            stop=(kd == KD - 1),
                        )
                    ko = kout.tile([P, 512], BF16, tag="ko")
                    nc.vector.tensor_copy(ko, ps)
                    nc.sync.dma_start(kt_stage[f, :, n * 512:(n + 1) * 512], ko)

        # ---------------- Q projection: QT_all[F, Sq] -> SBUF --------------
        wq = wpool.tile([P, KD, F], BF16, tag="w")
        nc.sync.dma_start(wq, ins["wqT"].rearrange("(c p) f -> p c f", p=P))
        with (
            tc.tile_pool(name="xq", bufs=1) as xpool,
            tc.tile_pool(name="qps", bufs=4, space="PSUM") as qps,
        ):
            xq = xpool.tile([P, KD, SQ], BF16)
            nc.sync.dma_start(xq, xq_r)
            for f in range(NF):  # chunk-major so qt_sb[:, 0] is ready first
                for n in range(SQ // 512):
                    ps = qps.tile([P, 512], F32, tag="ps")
                    for kd in range(KD):
                        nc.tensor.matmul(
                            ps,
                            lhsT=wq[:, kd, f * P:(f + 1) * P],
                            rhs=xq[:, kd, n * 512:(n + 1) * 512],
                            start=(kd == 0),
                            stop=(kd == KD - 1),
                        )
                    nc.vector.tensor_copy(qt_sb[:, f, n * 512:(n + 1) * 512], ps)

        # ---------------- attention, head by head --------------------------
        with (
            tc.tile_pool(name="ktc", bufs=2) as ktp,
            tc.tile_pool(name="expp", bufs=2) as expp,
            tc.tile_pool(name="rcp", bufs=2) as rcp,
            tc.tile_pool(name="bcs", bufs=2) as bcs,
            tc.tile_pool(name="oto", bufs=2) as oto,
            tc.tile_pool(name="scps", bufs=2, space="PSUM") as scps,
            tc.tile_pool(name="smps", bufs=4, space="PSUM") as smps,
        ):
            for c in range(NF):  # head-pair chunks
                ktc = ktp.tile([P, SK], BF16, tag="kt")
                nc.sync.dma_start(ktc, kt_stage[c])
                for sq in range(2):
                    # Both heads of the pair accumulate together; their
                    # score matmuls occupy disjoint PE row groups (rows
                    # 0-63 / 64-127) and run concurrently. Emission is
                    # software-pipelined: scores for chunk sk+1 are issued
                    # BEFORE the PV matmuls of chunk sk, so the PE computes
                    # next scores while ACT runs exp(sk) instead of
                    # stalling behind the exp-dependent PV.
                    ot_ps = [
                        smps.tile([65, 512], F32, tag="sm", name="otp")
                        for _ in range(2)
                    ]

                    def emit_scores(sk):
                        sc = scps.tile([P, 2, 512], F32, tag="sc", name="sc")
                        for hh in range(2):
                            base = hh * 64
                            nc.tensor.matmul(
                                sc[:, hh, :],
                                lhsT=ktc[base:base + 64, sk * P:(sk + 1) * P],
                                rhs=qt_sb[base:base + 64, c,
                                          sq * 512:(sq + 1) * 512],
                                start=True,
                                stop=True,
                            )
                        return sc

                    sc_prev = emit_scores(0)
                    for sk in range(NSK):
                        ex = expp.tile([P, 2, 512], BF16, tag="ex", name="ex")
                        nc.scalar.activation(ex, sc_prev, AF.Exp,
                                             scale=1.0 / TEMP)
                        if sk + 1 < NSK:
                            sc_prev = emit_scores(sk + 1)
                        for hh in range(2):
                            nc.tensor.matmul(
                                ot_ps[hh],
                                lhsT=v_sb[:, sk, 2 * c + hh, :],
                                rhs=ex[:, hh, :],
                                start=(sk == 0),
                                stop=(sk == NSK - 1),
                            )
                    for hh in range(2):
                        base = hh * 64
                        # row 64 of ot_ps = softmax denominator
                        rc = rcp.tile([65, 512], F32, tag="rc")
                        nc.vector.reciprocal(rc[64:65, :], ot_ps[hh][64:65, :])
                        rcb = rcp.tile([65, 512], BF16, tag="rcb")
                        nc.vector.tensor_copy(rcb[64:65, :], rc[64:65, :])
                        bc_ps = smps.tile([64, 512], F32, tag="sm")
                        nc.tensor.matmul(
                            bc_ps,
                            lhsT=ones_sb[64:65, 0:64],
                            rhs=rcb[64:65, :],
                            start=True,
                            stop=True,
                        )
                        bc = bcs.tile([64, 512], F32, tag="bc")
                        nc.vector.tensor_copy(bc, bc_ps)
                        oo = oto.tile([64, 512], BF16, tag="oo")
                        nc.vector.tensor_mul(oo, ot_ps[hh][0:64, :], bc)
                        nc.sync.dma_start(
                            ot_stage[c, base:base + 64, sq * 512:(sq + 1) * 512],
                            oo,
                        )

    # ---------------- output projection + residual + layernorm -------------
    with (
        tc.tile_pool(name="wp", bufs=1) as wpp,
        tc.tile_pool(name="lnc", bufs=1) as lnc,
        tc.tile_pool(name="otf", bufs=16) as otf,
        tc.tile_pool(name="qres", bufs=3) as qrp,
        tc.tile_pool(name="lnw", bufs=4) as lnw,
        tc.tile_pool(name="stat", bufs=8) as stp,
        tc.tile_pool(name="fps", bufs=3, space="PSUM") as fps,
    ):
        wp = wpp.tile([P, NF, D], BF16)
        nc.sync.dma_start(wp, ins["wpT"].rearrange("(c p) f -> p c f", p=P))
        scale_sb = lnc.tile([P, 2, 512], F32)
        nc.sync.dma_start(scale_sb, ins["scale_b"].rearrange("p (a b) -> p a b", a=2))
        offset_sb = lnc.tile([P, 2, 512], F32)
        nc.sync.dma_start(offset_sb, ins["offset_b"].rearrange("p (a b) -> p a b", a=2))

        for sq in range(SQ // P):  # 8 query chunks of 128
            ots = []
            for f in range(NF):
                t = otf.tile([P, P], BF16, tag="ot", name="ott")
                nc.sync.dma_start(t, ot_stage[f, :, sq * P:(sq + 1) * P])
                ots.append(t)
            qr = qrp.tile([P, 2, 512], F32, tag="qr")
            nc.sync.dma_start(
                qr,
                ins["qres"][sq * P:(sq + 1) * P, :].rearrange(
                    "p (a b) -> p a b", a=2),
            )
            fp = fps.tile([P, 2, 512], F32, tag="fp")
            for d in range(2):
                for f in range(NF):
                    nc.tensor.matmul(
                        fp[:, d, :],
                        lhsT=ots[f],
                        rhs=wp[:, f, d * 512:(d + 1) * 512],
                        start=(f == 0),
                        stop=(f == NF - 1),
                    )
            x = lnw.tile([P, 2, 512], F32, tag="x")
            nc.vector.tensor_add(x, fp, qr)
            stats = stp.tile([P, 2, 6], F32, tag="st")
            for gsub in range(2):
                nc.vector.bn_stats(stats[:, gsub, :], x[:, gsub, :])
            mv = stp.tile([P, 2], F32, tag="mv")
            nc.vector.bn_aggr(mv, stats)
            # unbiased std + eps, then reciprocal
            stdt = stp.tile([P, 1], F32, tag="sd")
            nc.scalar.activation(stdt, mv[:, 1:2], AF.Sqrt,
                                 scale=float(D) / float(D - 1))
            nc.vector.tensor_scalar_add(stdt, stdt, EPS)
            rstd = stp.tile([P, 1], F32, tag="rs")
            nc.vector.reciprocal(rstd, stdt)
            xn = lnw.tile([P, 2, 512], F32, tag="xn")
            nc.vector.tensor_scalar(xn, x, mv[:, 0:1], rstd,
                                    ALU.subtract, ALU.mult)
            nc.gpsimd.tensor_mul(xn, xn, scale_sb)
            nc.gpsimd.tensor_add(xn, xn, offset_sb)
            nc.sync.dma_start(
                out_ap[sq * P:(sq + 1) * P, :],
                xn.rearrange("p a b -> p (a b)"),
            )


def build_program():
    nc = bacc.Bacc("TRN2", debug=False, target_bir_lowering=False)
    shapes = {
        "qT": ([D, SQ], BF16), "kT": ([D, SK], BF16), "vT": ([D, SK], BF16),
        "qres": ([SQ, D], F32),
        "wqT": ([D, F], BF16), "wkT": ([D, F], BF16), "wvT": ([D, F], BF16),
        "wpT": ([F, D], BF16),
        "scale_b": ([P, D], F32), "offset_b": ([P, D], F32),
    }
    ins = {k: nc.dram_tensor(k, shp, dt, kind="ExternalInput").ap()
           for k, (shp, dt) in shapes.items()}
    out = nc.dram_tensor("out", [SQ, D], F32, kind="ExternalOutput").ap()
    with tile.TileContext(nc) as tc:
        _mha_kernel(tc, out, ins)
    nc.compile()
    return nc


_PROGRAM = None


def _get_program():
    global _PROGRAM
    if _PROGRAM is None:
        _PROGRAM = build_program()
    return _PROGRAM


def make_in_maps(q, k, v, Wq, Wk, Wv, Wp, scale, offset):
    import ml_dtypes
    f = np.float32
    bf = ml_dtypes.bfloat16
    q = np.asarray(q, f)
    k16 = np.asarray(k, f).astype(bf)
    v16 = np.asarray(v, f).astype(bf)
    q16 = q.astype(bf)
    wqT = np.ascontiguousarray(
        np.asarray(Wq, f).transpose(2, 0, 1).reshape(D, F).astype(bf))
    wkT = np.ascontiguousarray(
        np.asarray(Wk, f).transpose(2, 0, 1).reshape(D, F).astype(bf))
    wvT = np.ascontiguousarray(
        np.asarray(Wv, f).transpose(2, 0, 1).reshape(D, F).astype(bf))
    wpT = np.ascontiguousarray(np.asarray(Wp, f).T.astype(bf))
    scale_b = np.ascontiguousarray(
        np.broadcast_to(np.asarray(scale, f), (P, D)))
    offset_b = np.ascontiguousarray(
        np.broadcast_to(np.asarray(offset, f), (P, D)))
    in_maps = []
    for c in range(N_CORES):
        b, half = divmod(c, 2)
        sl = slice(half * SQ, (half + 1) * SQ)
        in_maps.append({
            "qT": np.ascontiguousarray(q16[b, sl].T),
            "qres": np.ascontiguousarray(q[b, sl]),
            "kT": np.ascontiguousarray(k16[b].T),
            "vT": np.ascontiguousarray(v16[b].T),
            "wqT": wqT, "wkT": wkT, "wvT": wvT, "wpT": wpT,
            "scale_b": scale_b, "offset_b": offset_b,
        })
    return in_maps


def kernel(q, k, v, Wq, Wk, Wv, Wp, scale, offset):
    global LAST_RESULT
    in_maps = make_in_maps(q, k, v, Wq, Wk, Wv, Wp, scale, offset)
    nc = _get_program()
    res = run_bass_kernel_spmd(nc, in_maps, list(range(N_CORES)))
    LAST_RESULT = res
    out = np.empty((B, S, D), np.float32)
    for c in range(N_CORES):
        b, half = divmod(c, 2)
        out[b, half * SQ:(half + 1) * SQ] = res.results[c]["out"]
    return out
